# revision 16
# baseline (speedup 1.0000x reference)
"""Causal self-attention (B=4, T=2048, C=1024, H=16, D=64) on 8 TRN2 NeuronCores.

Sharding: core = 2*b + g  (b = batch 0..3, g = head-group 0..1; heads 8g..8g+7).
Each core computes, for its batch b and its 8 heads:
  qkv projection, causal softmax attention, and a PARTIAL output projection
  (its 512 rows of W_proj). Host sums the two partials per batch and adds
  b_proj plus the v-bias contribution b_qkv[2048:] @ W_proj (the v-bias
  commutes through the softmax-normalized attention average).

v3 precision/layout (error budget measured per stage; gate is 2e-2):
  - Projections in bf16 (plain matmul, same PE rate as f32r; fp8 products in
    the projections leave too much noise at softmax-peaked rows).
  - qt/kt stored bf16 [128 (2 heads x 64 d), T]; q-bias folded into the
    psum->SBUF copy; k-bias dropped (softmax shift invariance).
  - scores S^T = K.T @ Q in PSUM f32; causal diag masked by accumulating a
    rank-128 fp8 matmul tri8.T @ id8 (= -240 above the diagonal) into the
    same PSUM group; exp on ACT (scale 1/8, bias -3.5 to fit the fp8 range)
    writes P^T to SBUF as fp8 e4m3.
  - V stored as e4m3 hi + lo split ([128 keys, 8h, 2 ki-slots, 64 d] x2);
    AV runs two DoubleRow chains (0.5 cyc/col, two key-tiles per matmul)
    plus an M=32 DoubleRow ones-matmul for the softmax denominator.
  - normalize: DVE reciprocal + GPSIMD partition_broadcast + DVE multiply
    -> yt [128 (2h x 64), T] f32r; out projection in f32r.
  - AV is interleaved per key-pair into the scores/exp loop so the Tensor
    engine fills the Act-engine latency instead of serializing.
"""

import sys

try:
    import concourse  # noqa: F401
except ImportError:
    sys.path.insert(0, "/opt/trn_rl_repo")

import numpy as np

import concourse.bacc as bacc
import concourse.mybir as mybir
import concourse.tile as tile

F32 = mybir.dt.float32
F32R = mybir.dt.float32r
F8 = mybir.dt.float8e4
BF16 = mybir.dt.bfloat16
AF = mybir.ActivationFunctionType
DR = mybir.MatmulPerfMode.DoubleRow

B, T, C = 4, 2048, 1024
H, D = 16, 64
NCORES = 8
HL = 8          # heads per core
NPAIR = 4       # head pairs per core
CH = 1024       # q chunk (PSUM-sized)
NCH = T // CH   # 2
KT = T // 128   # 16 key tiles
CT = C // 128   # 8 contraction tiles
SCALE = 1.0 / 8.0   # 1/sqrt(D)
EBIAS = -2.7        # exp bias: keeps max exp(s/8+EBIAS) < 240 (e4m3 max)

_prog_cache = {}


def _av_ops(c):
    """AV op list for chunk c, ordered by key-pair jp then column chunk.
    Returns [(jp, kind, a, b, st, sp)]: kind 'sh' reads both ki slots
    (DoubleRow), 'st' reads slot 0 only; st/sp are the accumulation-group
    start/stop flags, assigned per 512-col PSUM zero region."""
    njp = 4 * (c + 1)
    ops = []
    for jp in range(njp):
        q0 = max(0, 256 * jp - CH * c)
        q1 = max(0, 128 * (2 * jp + 1) - CH * c)
        if q1 > q0:
            ops.append([jp, "st", q0, q1])
        a = q1
        while a < CH:
            b = min(a + 256 - a % 256, CH)
            ops.append([jp, "sh", a, b])
            a = b
    flags = []
    for i, (jp, kind, a, b) in enumerate(ops):
        r = a // 512
        same = [j for j, o in enumerate(ops) if o[2] // 512 == r]
        flags.append((jp, kind, a, b, i == same[0], i == same[-1]))
    return flags


def build_program(debug=False):
    key = debug
    if key in _prog_cache:
        return _prog_cache[key]

    nc = bacc.Bacc(None, target_bir_lowering=False)
    dump = debug

    xtb = nc.dram_tensor("xtb", [128, CT, T], BF16, kind="ExternalInput")
    wqb = nc.dram_tensor("wqb", [128, CT, 512], BF16, kind="ExternalInput")
    wkb = nc.dram_tensor("wkb", [128, CT, 512], BF16, kind="ExternalInput")
    wvb = nc.dram_tensor("wvb", [128, CT, 512], BF16, kind="ExternalInput")
    bq = nc.dram_tensor("bq", [128, 4], F32, kind="ExternalInput")
    tri8 = nc.dram_tensor("tri8", [128, 128], F8, kind="ExternalInput")
    id8 = nc.dram_tensor("id8", [128, 128], F8, kind="ExternalInput")
    ones8 = nc.dram_tensor("ones8", [128, 64], F8, kind="ExternalInput")
    wp = nc.dram_tensor("wp", [512, C], F32R, kind="ExternalInput")
    out = nc.dram_tensor("out", [T, C], F32, kind="ExternalOutput")
    if dump:
        d_qt = nc.dram_tensor("d_qt", [128, T], BF16, kind="ExternalOutput")
        d_kt = nc.dram_tensor("d_kt", [128, T], BF16, kind="ExternalOutput")
        d_vh = nc.dram_tensor("d_vh", [128, HL, 2, 64], F8, kind="ExternalOutput")
        d_vl = nc.dram_tensor("d_vl", [128, HL, 2, 64], F8, kind="ExternalOutput")
        d_pt = nc.dram_tensor("d_pt", [128, 2, CH], F8, kind="ExternalOutput")
        d_yt = nc.dram_tensor("d_yt", [128, T], F32, kind="ExternalOutput")

    with tile.TileContext(nc) as tc:
        with (
            tc.tile_pool(name="consts", bufs=1) as consts,
            tc.tile_pool(name="xtp", bufs=1) as xtp,
            tc.tile_pool(name="wqkp", bufs=1) as wqkp,
            tc.tile_pool(name="vp", bufs=1) as vp,
            tc.tile_pool(name="qkt", bufs=1) as qkt,
            tc.tile_pool(name="ytp", bufs=1) as ytp,
            tc.tile_pool(name="ptp", bufs=10) as ptp,
            tc.tile_pool(name="nrm", bufs=2) as nrm,
            tc.tile_pool(name="ps", bufs=2, space="PSUM") as ps,
        ):
            # ---- constants
            tri_sb = consts.tile([128, 128], F8, tag="tri")
            id_sb = consts.tile([128, 128], F8, tag="id")
            ones_sb = consts.tile([128, 64], F8, tag="ones")
            bq_sb = consts.tile([128, 4], F32, tag="bq")
            for dst, src in ((tri_sb, tri8), (id_sb, id8), (ones_sb, ones8),
                             (bq_sb, bq)):
                nc.sync.dma_start(out=dst, in_=src.ap())
            dones = ones_sb.rearrange("p (i m) -> p i m", i=2)  # [128,2,32]
            eb_sb = consts.tile([128, 1], F32, tag="ebias")
            nc.vector.memset(eb_sb, EBIAS)

            # ---- resident xt [128, 8, T] bf16, loaded in t-chunks
            xt_sb = xtp.tile([128, CT, T], BF16, tag="xt")
            for (c0, c1) in ((0, 256), (256, 1024), (1024, 2048)):
                nc.sync.dma_start(out=xt_sb[:, :, c0:c1],
                                  in_=xtb.ap()[:, :, c0:c1])

            wv_sb = wqkp.tile([128, CT, 512], BF16, tag="wv")
            wq_sb = wqkp.tile([128, CT, 512], BF16, tag="wq")
            wk_sb = wqkp.tile([128, CT, 512], BF16, tag="wk")
            nc.sync.dma_start(out=wv_sb, in_=wvb.ap())
            nc.sync.dma_start(out=wq_sb, in_=wqb.ap())
            nc.sync.dma_start(out=wk_sb, in_=wkb.ap())

            # ---- V projection (bf16) -> v_hi + v_lo e4m3 [128, 8h, 2sl, 64]
            vh_sb, vl_sb = [], []
            for jp in range(KT // 2):
                vh_sb.append(vp.tile([128, HL, 2, 64], F8, tag=f"vh{jp}",
                                     name=f"vh{jp}"))
                vl_sb.append(vp.tile([128, HL, 2, 64], F8, tag=f"vl{jp}",
                                     name=f"vl{jp}"))
            for ki in range(KT):
                jp, sl = ki // 2, ki % 2
                pv = ps.tile([128, 512], F32, tag="stp", bufs=2,
                             name=f"pv{ki}")
                for k in range(CT):
                    nc.tensor.matmul(
                        pv,
                        lhsT=xt_sb[:, k, 128 * ki:128 * ki + 128],
                        rhs=wv_sb[:, k, :],
                        start=(k == 0), stop=(k == CT - 1),
                    )
                pv_r = pv.rearrange("p (h d) -> p h d", h=HL)
                nc.vector.tensor_copy(vh_sb[jp][:, :, sl, :], pv_r)
                nc.vector.tensor_sub(vl_sb[jp][:, :, sl, :], pv_r,
                                     vh_sb[jp][:, :, sl, :])

            # ---- QK projection (bf16) into bf16 qt/kt pair tiles
            qt_sb, kt_sb = [], []
            for pr in range(NPAIR):
                qt_sb.append(qkt.tile([128, T], BF16, tag=f"qt{pr}",
                                      name=f"qt{pr}"))
                kt_sb.append(qkt.tile([128, T], BF16, tag=f"kt{pr}",
                                      name=f"kt{pr}"))
            yt_sb = []
            for pr in range(NPAIR):
                yt_sb.append(ytp.tile([128, T], F32R, tag=f"yt{pr}",
                                      name=f"yt{pr}"))

            def emit_qk_proj(pr):
                for side, (dsts, wsb) in enumerate(
                        ((qt_sb, wq_sb), (kt_sb, wk_sb))):
                    for c2 in range(4):  # t chunks of 512
                        pq = ps.tile([128, 512], F32, tag="stp", bufs=2,
                                     name=f"pq{pr}_{side}_{c2}")
                        for k in range(CT):
                            nc.tensor.matmul(
                                pq,
                                lhsT=wsb[:, k, 128 * pr:128 * pr + 128],
                                rhs=xt_sb[:, k, 512 * c2:512 * c2 + 512],
                                start=(k == 0), stop=(k == CT - 1),
                            )
                        if side == 0:  # fold q-bias into the copy
                            nc.vector.tensor_scalar(
                                out=dsts[pr][:, 512 * c2:512 * c2 + 512],
                                in0=pq, scalar1=bq_sb[:, pr:pr + 1],
                                scalar2=None, op0=mybir.AluOpType.add,
                            )
                        else:
                            nc.vector.tensor_copy(
                                dsts[pr][:, 512 * c2:512 * c2 + 512], pq)

            def emit_attention(pr, hh, c):
                hl = 2 * pr + hh
                base = 64 * hh
                qt, kt = qt_sb[pr], kt_sb[pr]
                nk = 8 * (c + 1)
                ytps = ps.tile([64, CH], F32, tag="ytps", bufs=1,
                               name=f"ytps{hl}_{c}")
                denps = ps.tile([32, CH], F32, tag="denps", bufs=1,
                                name=f"denps{hl}_{c}")
                pts = []
                for jp in range(nk // 2):
                    pts.append(ptp.tile([128, 2, CH], F8, tag="pt",
                                        name=f"pt{hl}_{c}_{jp}"))
                av_by_jp = {}
                for (jp, kind, a, b, st_, sp_) in _av_ops(c):
                    av_by_jp.setdefault(jp, []).append((kind, a, b, st_, sp_))

                for ki in range(nk):
                    jp, sl = ki // 2, ki % 2
                    q_off = max(0, 128 * ki - CH * c)
                    diag = ki >= 8 * c
                    stp = ps.tile([128, CH], F32, tag="stp", bufs=2,
                                  name=f"stp{hl}_{c}_{ki}")
                    ops = []
                    if q_off < 512:
                        ops.append((q_off, 512, False))
                    ops.append((max(q_off, 512), CH, False))
                    if diag:
                        ops.append((q_off, q_off + 128, True))
                    ops = [o for o in ops if o[0] < o[1]]
                    for r in (0, 1):
                        rops = [o for o in ops
                                if o[0] >= 512 * r and o[1] <= 512 * (r + 1)]
                        for oi, (s0, s1, m) in enumerate(rops):
                            st_, sp_ = oi == 0, oi == len(rops) - 1
                            if m:
                                nc.tensor.matmul(
                                    stp[:, s0:s1], lhsT=tri_sb, rhs=id_sb,
                                    start=st_, stop=sp_,
                                )
                            else:
                                nc.tensor.matmul(
                                    stp[:, s0:s1],
                                    lhsT=kt[base:base + 64,
                                            128 * ki:128 * ki + 128],
                                    rhs=qt[base:base + 64,
                                           CH * c + s0:CH * c + s1],
                                    start=st_, stop=sp_,
                                )
                    nc.scalar.activation(
                        out=pts[jp][:, sl, q_off:CH],
                        in_=stp[:, q_off:CH],
                        func=AF.Exp, scale=SCALE, bias=eb_sb[:, 0:1],
                    )
                    if sl == 1:  # AV for this key pair, fills Act latency
                        for (kind, a, b, st_, sp_) in av_by_jp[jp]:
                            if kind == "sh":
                                nc.tensor.matmul(
                                    ytps[:, a:b],
                                    lhsT=vh_sb[jp][:, hl, :, :],
                                    rhs=pts[jp][:, :, a:b],
                                    start=st_, stop=False, perf_mode=DR,
                                )
                                nc.tensor.matmul(
                                    ytps[:, a:b],
                                    lhsT=vl_sb[jp][:, hl, :, :],
                                    rhs=pts[jp][:, :, a:b],
                                    start=False, stop=sp_, perf_mode=DR,
                                )
                                nc.tensor.matmul(
                                    denps[:, a:b], lhsT=dones,
                                    rhs=pts[jp][:, :, a:b],
                                    start=st_, stop=sp_, perf_mode=DR,
                                )
                            else:
                                nc.tensor.matmul(
                                    ytps[:, a:b],
                                    lhsT=vh_sb[jp][:, hl, 0, :],
                                    rhs=pts[jp][:, 0, a:b],
                                    start=st_, stop=False,
                                )
                                nc.tensor.matmul(
                                    ytps[:, a:b],
                                    lhsT=vl_sb[jp][:, hl, 0, :],
                                    rhs=pts[jp][:, 0, a:b],
                                    start=False, stop=sp_,
                                )
                                nc.tensor.matmul(
                                    denps[:, a:b], lhsT=ones_sb[:, 0:32],
                                    rhs=pts[jp][:, 0, a:b],
                                    start=st_, stop=sp_,
                                )
                # normalize: yt = ytps * (1/den)
                rcp1 = nrm.tile([1, CH], F32, tag="rcp", name=f"rcp{hl}_{c}")
                nc.vector.reciprocal(out=rcp1, in_=denps[0:1, :])
                rb = nrm.tile([64, CH], F32, tag="rb", name=f"rb{hl}_{c}")
                nc.gpsimd.partition_broadcast(rb, rcp1)
                nc.vector.tensor_mul(
                    yt_sb[pr][base:base + 64, CH * c:CH * (c + 1)], ytps, rb)

            # ---- emission: qk(0), then attention interleaved with later qk
            emit_qk_proj(0)
            for pr in range(NPAIR):
                if pr + 1 < NPAIR:
                    emit_qk_proj(pr + 1)
                for hh in range(2):
                    emit_attention(pr, hh, 0)
            for pr in range(NPAIR):
                for hh in range(2):
                    emit_attention(pr, hh, 1)

            if dump:
                nc.sync.dma_start(out=d_qt.ap(), in_=qt_sb[0])
                nc.sync.dma_start(out=d_kt.ap(), in_=kt_sb[0])
                nc.sync.dma_start(out=d_vh.ap(), in_=vh_sb[0])
                nc.sync.dma_start(out=d_vl.ap(), in_=vl_sb[0])
                nc.sync.dma_start(out=d_yt.ap(), in_=yt_sb[0].bitcast(F32))

            # ---- output projection: out[qtile, :] = sum_p ytT.T @ wp_rows
            with (
                tc.tile_pool(name="wpp", bufs=1) as wpp,
                tc.tile_pool(name="outp", bufs=2) as outp,
            ):
                wp_r = wp.ap().rearrange("(k p) n -> k p n", p=128)
                wp_sb = []
                for k in range(NPAIR):
                    t_ = wpp.tile([128, C], F32R, tag=f"wp{k}")
                    nc.sync.dma_start(out=t_, in_=wp_r[k])
                    wp_sb.append(t_)

                for qt_i in range(KT):
                    pso = [
                        ps.tile([128, 512], F32, tag="stp", bufs=2,
                                name=f"pso{qt_i}_0"),
                        ps.tile([128, 512], F32, tag="stp", bufs=2,
                                name=f"pso{qt_i}_1"),
                    ]
                    for pr in range(NPAIR):
                        for nch in range(2):
                            nc.tensor.matmul(
                                pso[nch],
                                lhsT=yt_sb[pr][:, qt_i * 128:(qt_i + 1) * 128],
                                rhs=wp_sb[pr][:, nch * 512:(nch + 1) * 512],
                                start=(pr == 0), stop=(pr == NPAIR - 1),
                            )
                    ot = outp.tile([128, C], F32, tag="ot")
                    for nch in range(2):
                        nc.vector.tensor_copy(
                            ot[:, nch * 512:(nch + 1) * 512], pso[nch]
                        )
                    nc.sync.dma_start(
                        out=out.ap()[qt_i * 128:(qt_i + 1) * 128, :], in_=ot
                    )

    nc.compile()
    _prog_cache[key] = nc
    return nc


def _to_bf16(a):
    from ml_dtypes import bfloat16
    return np.ascontiguousarray(a).astype(bfloat16)


def _to_fp8(a):
    from ml_dtypes import float8_e4m3
    return np.ascontiguousarray(a).astype(float8_e4m3).view(np.uint8)


def shard_inputs(x, W_qkv, b_qkv, W_proj, core):
    b, g = core // 2, core % 2
    # xtb[p, j, t] = x[b, t, 128j + p]
    xt = x[b].T.reshape(CT, 128, T).transpose(1, 0, 2)

    def wslice(col0):  # [p, k, n] = W_qkv[128k + p, col0 + n]
        return W_qkv[:, col0:col0 + 512].reshape(CT, 128, 512).transpose(1, 0, 2)

    tri = np.where(np.arange(128)[None, :] > np.arange(128)[:, None],
                   np.float32(-240.0), np.float32(0.0))
    # q-bias as per-partition column per pair: bq[p, pr] with p = 2h*64+d
    bqv = b_qkv[512 * g:512 * g + 512].reshape(4, 128).T  # [128, 4 pairs]
    return {
        "xtb": _to_bf16(xt),
        "wqb": _to_bf16(wslice(512 * g)),
        "wkb": _to_bf16(wslice(1024 + 512 * g)),
        "wvb": _to_bf16(wslice(2048 + 512 * g)),
        "bq": np.ascontiguousarray(bqv).astype(np.float32),
        "tri8": _to_fp8(tri),
        "id8": _to_fp8(np.eye(128, dtype=np.float32)),
        "ones8": _to_fp8(np.ones((128, 64), np.float32)),
        "wp": np.ascontiguousarray(W_proj[512 * g:512 * g + 512, :]),
    }


def kernel(x, W_qkv, b_qkv, W_proj, b_proj, **run_kwargs):
    x = np.asarray(x, np.float32)
    W_qkv = np.asarray(W_qkv, np.float32)
    b_qkv = np.asarray(b_qkv, np.float32)
    W_proj = np.asarray(W_proj, np.float32)
    b_proj = np.asarray(b_proj, np.float32)

    nc = build_program()
    in_maps = [
        shard_inputs(x, W_qkv, b_qkv, W_proj, core) for core in range(NCORES)
    ]
    from concourse.bass_utils import run_bass_kernel_spmd

    res = run_bass_kernel_spmd(nc, in_maps, core_ids=list(range(NCORES)), **run_kwargs)
    outs = [r["out"] for r in res.results]
    # v-bias commutes through softmax-normalized attention: y += b_v,
    # handled as a constant output row (plus b_proj).
    const_row = (b_proj + b_qkv[2048:] @ W_proj).astype(np.float32)
    full = np.stack([outs[2 * b_] + outs[2 * b_ + 1] + const_row
                     for b_ in range(B)])
    kernel.last_results = res
    return full
